# revision 1
# baseline (speedup 1.0000x reference)
"""Trainium2 Bass kernel for nn_MemoryReader.

Reference computation (per batch b):
    mi = mk.reshape(CK, N);  qi = qk.reshape(CK, P) / sqrt(CK)
    S  = mi.T @ qi                      # [N, P] affinity logits
    A  = softmax(S, axis=0)             # over memory axis N
    mem = mv.reshape(CV, N) @ A         # [CV, P]
    out = concat([mem, qv], axis=channel)

Sharding: 8 cores = (4 batches) x (2 halves of the memory axis N).
Each core computes, for its (b, half):
    E      = exp(S_half)                        # no max subtraction (logits ~ N(0,1))
    memT   = E.T @ mv_half.T                    # [P, CV] unnormalized numerator
    lsum   = ones @ E                           # [1, P] denominator part
The host combines: mem = (mem_un_0 + mem_un_1) / (lsum_0 + lsum_1), then
concats qv (pure passthrough). No on-device collectives needed.

Device layout notes:
  - E is produced directly in [n(partition), p(free)] layout by computing
    S = mk_tile.T @ qk (lhsT = mk slice, K=CK=64 on partitions).
  - The second matmul contracts over n, so both operands need n on
    partitions: mv is pre-transposed ON THE HOST into [128, NT, CV]
    (partition-major tiles), making the device program transpose-free.
  - 1/sqrt(CK) is folded into the exp activation's free affine scale.
"""

import numpy as np
import ml_dtypes

import concourse.tile as tile
from concourse import bacc, mybir
from concourse.bass_utils import run_bass_kernel_spmd

# Problem shape (hardcoded per contract)
B, CK, CV, T, H, W = 4, 64, 512, 8, 30, 54
N = T * H * W          # 12960 memory positions
P = H * W              # 1620 query positions
NHALF = N // 2         # 6480 per core
NT = (NHALF + 127) // 128   # 51 n-tiles (last has 80 rows)
NLAST = NHALF - (NT - 1) * 128  # 80
NPAD = NT * 128        # 6528
# p-axis chunking: chunks of <=512 (one PSUM bank for mm1 out), each chunk
# sliced into 128-wide pieces that serve as mm2 stationary weights. The
# small 84-wide chunk runs LAST: its ACT-bound low-PE-duty cadence overlaps
# the output-DMA tail. (ps, width, n_slices); global slice id = ps//128 + sl.
PCHUNKS = [(0, 512, 4), (512, 512, 4), (1024, 512, 4), (1536, 84, 1)]
NSL = 13

# Matmul precision mode: "bf16" (1 cyc/col), "f32r" (fp32 data, ~1 cyc/col
# at free>=256), "f32" (4 cyc/col).
MM_MODE = "bf16"

_CACHE = {}


def _mm_dtype():
    return {
        "bf16": mybir.dt.bfloat16,
        "f32r": mybir.dt.float32r,
        "f32": mybir.dt.float32,
    }[MM_MODE]


def _np_dtype():
    return ml_dtypes.bfloat16 if MM_MODE == "bf16" else np.float32


def _mm_ap(ap):
    """Operand view handed to the tensor engine."""
    return ap


def _f32view(ap):
    """float32 view for vector-engine reads (f32r is fp32 bits)."""
    if MM_MODE == "f32r":
        return ap.bitcast(mybir.dt.float32)
    return ap


def _build_program():
    dt = _mm_dtype()
    f32 = mybir.dt.float32
    # Bacc (not plain Bass): its compile() runs generate_event_semaphores,
    # which splits multi-wait sync_info onto EventSemaphore instructions
    # (TRN2 allows only one wait per regular instruction).
    nc = bacc.Bacc(None, target_bir_lowering=False, debug=False)

    # NOTE on mm1 structure: the contraction dim is CK=64, but mk/qk are
    # zero-padded to K=128 on the host. Matmul time is column-bound (K is
    # free), and only full-row (K=128) LDWEIGHTS go through the background
    # weight buffer — K=64 weight loads (and tile_position row-packed pairs,
    # which were tried) serialize ~200ns per matmul on the weight port.
    mk_d = nc.declare_dram_parameter("mk", [128, NT, 128], dt, isOutput=False)
    qk_d = nc.declare_dram_parameter("qk", [128, P], dt, isOutput=False)
    mvt_d = nc.declare_dram_parameter("mvT", [128, NT, CV], dt, isOutput=False)
    # outputs in transposed layout: memT[p, v]; lsum packed [row, slice] with
    # l[p] at row=p%128, slice=p//128
    mem_d = nc.declare_dram_parameter("memT", [P, CV], f32, isOutput=True)
    l_d = nc.declare_dram_parameter("lsum", [128, 2 * NSL], f32, isOutput=True)

    with tile.TileContext(nc) as tc:
        with (
            tc.tile_pool(name="singles", bufs=1) as singles,
            tc.tile_pool(name="epool", bufs=4) as epool,
            tc.tile_pool(name="opool", bufs=8) as opool,
            tc.tile_pool(name="olpool", bufs=2) as olpool,
            tc.tile_pool(name="rpool", bufs=2) as rpool,
            tc.tile_pool(name="spsum", bufs=3, space="PSUM") as spsum,
            tc.tile_pool(name="accpsum", bufs=4, space="PSUM") as accpsum,
            tc.tile_pool(name="lpsum", bufs=1, space="PSUM") as lpsum,
        ):
            # fp32 ones for the (tiny, fp32) cross-partition R sum matmuls
            ones32 = singles.tile([128, 2], f32, name="ones32")
            nc.vector.memset(ones32, 1.0)
            qk_sb = singles.tile([128, P], dt)
            mk_sb = singles.tile([128, NT, 128], dt)
            mvt_sb = singles.tile([128, NT, CV], dt)
            # interleave the loads in consumption order: qk slivers per chunk
            # (the first chunk's is tiny, so compute starts immediately), then
            # mk tiles, then mvT. Each weight-tile read depends on exactly one
            # DMA (avoids multi-sem wait explosion).
            for ps_, w_, _ in PCHUNKS:
                nc.sync.dma_start(
                    out=qk_sb[:, ps_:ps_ + w_], in_=qk_d[:, ps_:ps_ + w_]
                )
            nc.sync.dma_start(out=mk_sb[:, 0:13, :], in_=mk_d[:, 0:13, :])
            NTG = 3
            for g in range(0, 6, NTG):
                nc.sync.dma_start(
                    out=mvt_sb[:, g:g + NTG, :], in_=mvt_d[:, g:g + NTG, :]
                )
            for g in range(13, NT, 13):
                g1 = min(g + 13, NT)
                nc.sync.dma_start(out=mk_sb[:, g:g1, :], in_=mk_d[:, g:g1, :])
            for g in range(6, NT, NTG):
                g1 = min(g + NTG, NT)
                nc.sync.dma_start(
                    out=mvt_sb[:, g:g1, :],
                    in_=mvt_d[:, g:g1, :],
                )

            # Warm-up: full-size (M=K=128) matmuls on a memset tile, depending
            # on no DMA. They run while the input DMAs land, filling the
            # initial PE idle gap AND releasing the HAM clock throttle (~3.4us
            # of sustained activity; tiny-M matmuls don't count as PE-busy).
            warmw = singles.tile([128, 128], mybir.dt.bfloat16, name="warmw")
            nc.vector.memset(warmw, 1.0)
            warm = lpsum.tile([128, 128], f32, tag="lacc", name="warm")
            for _ in range(48):
                nc.tensor.matmul(
                    warm,
                    lhsT=warmw,
                    rhs=warmw,
                    start=True,
                    stop=True,
                )

            def issue_mm1(ps, w, nt, s_pool_tiles):
                nsz = 128 if nt < NT - 1 else NLAST
                s = spsum.tile([128, 512], f32, tag="s", name="s")
                nc.tensor.matmul(
                    s[:nsz, :w],
                    lhsT=_mm_ap(mk_sb[:, nt, :nsz]),
                    rhs=_mm_ap(qk_sb[:, ps:ps + w]),
                    start=True,
                    stop=True,
                )
                s_pool_tiles[nt] = s

            for ci, (ps, w, nsl) in enumerate(PCHUNKS):
                sl0 = ps // 128  # global slice index of chunk's first slice
                acc = []
                for sl in range(nsl):
                    acc.append(accpsum.tile([128, CV], f32, tag="acc", name="acc"))
                # Denominator: R = sum over n-tiles of E, accumulated
                # elementwise on the (otherwise idle) vector engine; the
                # cross-partition sum happens in ONE matmul per slice at chunk
                # end. This keeps the PE inner loop free of the extra
                # weight-load per slice (the l matmuls' LDWEIGHTS were a
                # weight-port bottleneck).
                r_sb = rpool.tile([128, 512], f32, tag="r", name="r")
                nc.vector.memset(r_sb[:, :w], 0.0)
                # one psum bank for all slices' [pw, 2] column pairs (N=2:
                # N=1 is invalid for f32r; M=1 matmuls don't count as PE-busy
                # for the HAM clock gate). Only the first matmul of the bank
                # uses start=True (whole-bank has_written clear); later
                # slices rely on per-element overwrite-when-bit-unset.
                lacc = lpsum.tile([128, 2 * NSL], f32, tag="lacc", name="lacc")

                s_tiles = {}
                issue_mm1(ps, w, 0, s_tiles)
                for nt in range(NT):
                    if nt + 1 < NT:
                        issue_mm1(ps, w, nt + 1, s_tiles)
                    nsz = 128 if nt < NT - 1 else NLAST
                    s_cur = s_tiles.pop(nt)
                    e_sb = epool.tile([128, 512], dt, tag="e", name="e")
                    nc.scalar.activation(
                        out=e_sb[:nsz, :w],
                        in_=s_cur[:nsz, :w],
                        func=mybir.ActivationFunctionType.Exp,
                        scale=0.125,  # 1/sqrt(CK)
                    )
                    nc.vector.tensor_add(
                        out=r_sb[:nsz, :w],
                        in0=r_sb[:nsz, :w],
                        in1=_f32view(e_sb[:nsz, :w]),
                    )
                    first, last = nt == 0, nt == NT - 1
                    for sl in range(nsl):
                        pw = min(128, w - sl * 128)
                        el = e_sb[:nsz, sl * 128:sl * 128 + pw]
                        nc.tensor.matmul(
                            acc[sl][:pw],
                            lhsT=_mm_ap(el),
                            rhs=_mm_ap(mvt_sb[:nsz, nt, :]),
                            start=first,
                            stop=last,
                        )

                # cross-partition sum of R -> l, one matmul per slice
                for sl in range(nsl):
                    pw = min(128, w - sl * 128)
                    gsl = sl0 + sl
                    nc.tensor.matmul(
                        lacc[:pw, 2 * gsl:2 * gsl + 2],
                        lhsT=r_sb[:, sl * 128:sl * 128 + pw],
                        rhs=ones32,
                        start=sl == 0,
                        stop=sl == nsl - 1,
                        skip_group_check=True,
                    )

                for sl in range(nsl):
                    pw = min(128, w - sl * 128)
                    o_sb = opool.tile([128, CV], f32, tag="o", name="o")
                    nc.vector.tensor_copy(out=o_sb[:pw], in_=acc[sl][:pw])
                    p0 = ps + sl * 128
                    nc.sync.dma_start(out=mem_d[p0:p0 + pw, :], in_=o_sb[:pw])
                ol_sb = olpool.tile([128, 2 * NSL], f32, tag="ol", name="ol")
                nc.vector.tensor_copy(
                    out=ol_sb[:, 2 * sl0:2 * (sl0 + nsl)],
                    in_=lacc[:, 2 * sl0:2 * (sl0 + nsl)],
                )
                nc.sync.dma_start(
                    out=l_d[:, 2 * sl0:2 * (sl0 + nsl)],
                    in_=ol_sb[:, 2 * sl0:2 * (sl0 + nsl)],
                )

    _strip_same_engine_waits(nc)
    nc.compile()
    return nc


def _strip_same_engine_waits(nc):
    """Drop redundant same-engine semaphore waits on ACT/PE compute
    instructions.

    Each engine executes its queue in order, so an ACTIVATE waiting on the
    Activation engine's own completion semaphore (a WAW slot-reuse guard Tile
    emits conservatively) is a no-op — but TRN2 instructions hold only ONE
    wait, so the extra wait forces generate_event_semaphores to insert a
    separate EVENT_SEMAPHORE instruction that serializes the engine queue
    (~0.6us each on the scalar engine). DVE is left alone: its chains include
    genuine same-engine RAW dependencies.
    """
    prefixes = {
        "EngineType.Activation": "Activation_",
        "EngineType.PE": "PE_",
    }
    kinds = (mybir.InstActivation, mybir.InstMatmult, mybir.InstLdweights)
    for fn in nc.m.functions:
        for blk in fn.blocks:
            for inst in blk.instructions:
                si = getattr(inst, "sync_info", None)
                if si is None or not si.on_wait or not isinstance(inst, kinds):
                    continue
                pref = prefixes.get(str(getattr(inst, "engine", None)))
                if pref is None:
                    continue
                kept = [w for w in si.on_wait
                        if not str(getattr(w, "ant_name", "")).startswith(pref)]
                if len(kept) != len(si.on_wait):
                    si.on_wait = kept


def _get_program():
    if "nc" not in _CACHE:
        _CACHE["nc"] = _build_program()
    return _CACHE["nc"]


def _make_in_maps(mk, mv, qk):
    npdt = _np_dtype()
    mkf = np.ascontiguousarray(mk.reshape(B, CK, N))
    mvf = np.ascontiguousarray(mv.reshape(B, CV, N))
    qkf = np.ascontiguousarray(qk.reshape(B, CK, P))
    in_maps = []
    for core in range(8):
        b, half = core // 2, core % 2
        n0, n1 = half * NHALF, (half + 1) * NHALF
        mk_c = mkf[b, :, n0:n1].astype(npdt)          # [64, 6480]
        # zero-pad the contraction dim to 128 (see mm1 note in _build_program)
        mk_t = np.zeros((128, NT, 128), dtype=npdt)
        mk_t[:CK].reshape(CK, NT * 128)[:, :NHALF] = mk_c
        qk_c = np.zeros((128, P), dtype=npdt)
        qk_c[:CK] = qkf[b].astype(npdt)
        mvt = np.zeros((NPAD, CV), dtype=npdt)
        mvt[:NHALF] = mvf[b, :, n0:n1].T
        # partition-major tiles: [128, NT, CV], elem (p, t, v) = mvT[t*128+p, v]
        mvt_c = np.ascontiguousarray(mvt.reshape(NT, 128, CV).transpose(1, 0, 2))
        in_maps.append({"mk": np.ascontiguousarray(mk_t),
                        "qk": np.ascontiguousarray(qk_c),
                        "mvT": mvt_c})
    return in_maps


def _run(mk, mv, qk, qv, trace=False, **spmd_kwargs):
    nc = _get_program()
    in_maps = _make_in_maps(mk, mv, qk)
    res = run_bass_kernel_spmd(nc, in_maps, list(range(8)), trace=trace, **spmd_kwargs)
    out = np.empty((B, 2 * CV, P), dtype=np.float32)
    for b in range(B):
        m0, l0 = res.results[2 * b]["memT"], res.results[2 * b]["lsum"]
        m1, l1 = res.results[2 * b + 1]["memT"], res.results[2 * b + 1]["lsum"]
        # memT is [P, CV]; lsum [128, 2*NSL]: l[p] at [p % 128, 2*(p // 128)]
        lv = (l0 + l1)[:, 0::2].T.reshape(-1)[:P]
        out[b, :CV] = ((m0 + m1) / lv[:, None]).T
        out[b, CV:] = qv[b].reshape(CV, P)
    return out.reshape(B, 2 * CV, H, W), res


def kernel(mk, mv, qk, qv):
    out, _ = _run(np.asarray(mk), np.asarray(mv), np.asarray(qk), np.asarray(qv))
    return out



# revision 4
# speedup vs baseline: 1.3909x; 1.3909x over previous
"""Trainium2 Bass kernel for nn_MemoryReader (fp8 DoubleRow mm2).

Reference computation (per batch b):
    mi = mk.reshape(CK, N);  qi = qk.reshape(CK, P) / sqrt(CK)
    S  = mi.T @ qi                      # [N, P] affinity logits
    A  = softmax(S, axis=0)             # over memory axis N
    mem = mv.reshape(CV, N) @ A         # [CV, P]
    out = concat([mem, qv], axis=channel)

Sharding: 8 cores = (4 batches) x (2 halves of the memory axis N).
Each core computes, for its (b, half):
    E      = exp(S_half/8 - ln4)                # constant offset cancels in softmax
    memT   = E.T @ mv_half.T                    # [P, CV] unnormalized numerator
    r2     = per-(partition,plane) partial sums of E   # [128, 2, P]
The host combines: lsum = sum_rows(r2_0 + r2_1), mem = (m0 + m1) / lsum,
then concats qv (pure passthrough). No on-device collectives needed.

Device layout notes (v2, fp8):
  - mm1 (bf16): S for an even/odd n-tile PAIR is written into one 2-bank
    PSUM tile S2[128, 2, 512] (each plane = exactly one bank, so each
    matmul stays within a bank).
  - One exp ACTIVATE per (pair, chunk) converts S2[:, :, :w] -> fp8e4m3
    e8[128, 2, w]: this IS the DoubleRow [Ki, Ko=2, M] stationary layout.
  - mm2 runs perf_mode=DoubleRow: contraction 256 rows/pass, halving PE
    time vs bf16. rhs is host-packed fp8 mvT[128, pair, 2, CV].
  - n is padded 6480 -> 6656 (52 tiles): pad rows have zero mk -> S=0 ->
    E=0.25 (finite), and zero mv rows -> no numerator pollution. The
    denominator adds slice the pads out ([:80] of pair 25 plane 0).
  - Denominator: DVE accumulates r2 += e8 per pair (fp8 read, 1x mode);
    r2 is DMA'd raw and the host does the final 256-row sum. No PE
    l-matmuls, no PSUM bank for them.
  - p-axis chunks of 384 (3 PSUM acc banks) + final 84; PSUM = 2*S2(2) +
    4 acc (3 live per chunk, 4th lets the next chunk start early) = 8.
"""

import math

import numpy as np
import ml_dtypes

import concourse.tile as tile
from concourse import bacc, mybir
from concourse.bass_utils import run_bass_kernel_spmd

# Problem shape (hardcoded per contract)
B, CK, CV, T, H, W = 4, 64, 512, 8, 30, 54
N = T * H * W          # 12960 memory positions
P = H * W              # 1620 query positions
NHALF = N // 2         # 6480 per core
NT = 52                # n-tiles of 128 (padded: 6656; real rows 6480)
NT2 = NT // 2          # 26 DoubleRow pairs
NPAD = NT * 128        # 6656
NLAST = NHALF - 50 * 128    # 80 real rows in tile 50; tile 51 all pad
# p-axis chunking: (ps, width, n_slices); 384 = 3 psum acc banks per chunk.
PCHUNKS = [(0, 384, 3), (384, 384, 3), (768, 384, 3), (1152, 384, 3),
           (1536, 84, 1)]
EXP_BIAS = -math.log(4.0)  # keeps E in fp8e4m3 range; cancels in softmax

_CACHE = {}


def _build_program():
    bf16 = mybir.dt.bfloat16
    f8 = mybir.dt.float8e4
    f32 = mybir.dt.float32
    nc = bacc.Bacc(None, target_bir_lowering=False, debug=False)

    mk_d = nc.declare_dram_parameter("mk", [128, NT, 128], bf16, isOutput=False)
    qk_d = nc.declare_dram_parameter("qk", [128, P], bf16, isOutput=False)
    mv8_d = nc.declare_dram_parameter("mv8", [128, NT2, 2, CV], f8, isOutput=False)
    # outputs: memT[p, v] (transposed numerator); r2 raw partial sums
    mem_d = nc.declare_dram_parameter("memT", [P, CV], f32, isOutput=True)
    l_d = nc.declare_dram_parameter("lsum", [128, 2, P], f32, isOutput=True)

    with tile.TileContext(nc) as tc:
        with (
            tc.tile_pool(name="singles", bufs=1) as singles,
            tc.tile_pool(name="epool", bufs=4) as epool,
            tc.tile_pool(name="opool", bufs=8) as opool,
            tc.tile_pool(name="rpool", bufs=2) as rpool,
            tc.tile_pool(name="spsum", bufs=2, space="PSUM") as spsum,
            tc.tile_pool(name="accpsum", bufs=4, space="PSUM") as accpsum,
        ):
            qk_sb = singles.tile([128, P], bf16)
            mk_sb = singles.tile([128, NT, 128], bf16)
            mv8_sb = singles.tile([128, NT2, 2, CV], f8)
            # interleave loads in consumption order: qk slivers per chunk,
            # first mk tiles, first mv8 pairs, then the rest.
            for ps_, w_, _ in PCHUNKS:
                nc.sync.dma_start(
                    out=qk_sb[:, ps_:ps_ + w_], in_=qk_d[:, ps_:ps_ + w_]
                )
            nc.sync.dma_start(out=mk_sb[:, 0:13, :], in_=mk_d[:, 0:13, :])
            for g in range(0, 6, 2):
                nc.sync.dma_start(
                    out=mv8_sb[:, g:g + 2, :, :], in_=mv8_d[:, g:g + 2, :, :]
                )
            for g in range(13, NT, 13):
                g1 = min(g + 13, NT)
                nc.sync.dma_start(out=mk_sb[:, g:g1, :], in_=mk_d[:, g:g1, :])
            for g in range(6, NT2, 4):
                g1 = min(g + 4, NT2)
                nc.sync.dma_start(
                    out=mv8_sb[:, g:g1, :, :], in_=mv8_d[:, g:g1, :, :]
                )

            # Warm-up: full-size matmuls on a memset tile, depending on no
            # DMA. They run while the input DMAs land, filling the initial
            # PE idle gap AND releasing the HAM clock throttle.
            warmw = singles.tile([128, 128], bf16, name="warmw")
            nc.vector.memset(warmw, 1.0)
            bias_sb = singles.tile([128, 1], f32, name="bias")
            nc.vector.memset(bias_sb, EXP_BIAS)
            warm = accpsum.tile([128, 128], f32, tag="acc", name="warm")
            for _ in range(48):
                nc.tensor.matmul(
                    warm,
                    lhsT=warmw,
                    rhs=warmw,
                    start=True,
                    stop=True,
                )

            def issue_mm1(ps, w, t, s_tiles):
                s2 = spsum.tile([128, 2, 512], f32, tag="s", name="s")
                for j in (0, 1):
                    nc.tensor.matmul(
                        s2[:, j, :w],
                        lhsT=mk_sb[:, 2 * t + j, :],
                        rhs=qk_sb[:, ps:ps + w],
                        start=True,
                        stop=True,
                    )
                s_tiles[t] = s2

            for ci, (ps, w, nsl) in enumerate(PCHUNKS):
                acc = []
                for sl in range(nsl):
                    acc.append(accpsum.tile([128, CV], f32, tag="acc", name="acc"))
                r2 = rpool.tile([128, 2, 512], f32, tag="r", name="r")

                s_tiles = {}
                issue_mm1(ps, w, 0, s_tiles)
                for t in range(NT2):
                    if t + 1 < NT2:
                        issue_mm1(ps, w, t + 1, s_tiles)
                    s2 = s_tiles.pop(t)
                    e8 = epool.tile([128, 2, 512], f8, tag="e", name="e")
                    nc.scalar.activation(
                        out=e8[:, :, :w],
                        in_=s2[:, :, :w],
                        func=mybir.ActivationFunctionType.Exp,
                        scale=0.125,  # 1/sqrt(CK)
                        bias=bias_sb[:, 0:1],
                    )
                    # denominator partial sums (pads excluded; pair 25 has
                    # only 80 real rows, all in plane 0)
                    if t == 0:
                        nc.vector.tensor_copy(
                            out=r2[:, :, :w], in_=e8[:, :, :w]
                        )
                    elif t < NT2 - 1:
                        nc.vector.tensor_add(
                            out=r2[:, :, :w], in0=r2[:, :, :w], in1=e8[:, :, :w]
                        )
                    else:
                        nc.vector.tensor_add(
                            out=r2[:NLAST, 0, :w],
                            in0=r2[:NLAST, 0, :w],
                            in1=e8[:NLAST, 0, :w],
                        )
                    first, last = t == 0, t == NT2 - 1
                    for sl in range(nsl):
                        pw = min(128, w - sl * 128)
                        nc.tensor.matmul(
                            acc[sl][:pw],
                            lhsT=e8[:, :, sl * 128:sl * 128 + pw],
                            rhs=mv8_sb[:, t, :, :],
                            start=first,
                            stop=last,
                            perf_mode=mybir.MatmulPerfMode.DoubleRow,
                        )

                # raw denominator out; host does the 256-row sum
                nc.sync.dma_start(out=l_d[:, :, ps:ps + w], in_=r2[:, :, :w])
                for sl in range(nsl):
                    pw = min(128, w - sl * 128)
                    o_sb = opool.tile([128, CV], f32, tag="o", name="o")
                    nc.any.tensor_copy(out=o_sb[:pw], in_=acc[sl][:pw])
                    p0 = ps + sl * 128
                    nc.sync.dma_start(out=mem_d[p0:p0 + pw, :], in_=o_sb[:pw])

    _strip_same_engine_waits(nc)
    nc.compile()
    return nc


def _strip_same_engine_waits(nc):
    """Drop redundant same-engine semaphore waits on ACT/PE compute
    instructions (see baseline notes: TRN2 instructions hold one wait;
    extra waits force serializing EventSemaphore instructions)."""
    prefixes = {
        "EngineType.Activation": "Activation_",
        "EngineType.PE": "PE_",
    }
    kinds = (mybir.InstActivation, mybir.InstMatmult, mybir.InstLdweights)
    for fn in nc.m.functions:
        for blk in fn.blocks:
            for inst in blk.instructions:
                si = getattr(inst, "sync_info", None)
                if si is None or not si.on_wait or not isinstance(inst, kinds):
                    continue
                pref = prefixes.get(str(getattr(inst, "engine", None)))
                if pref is None:
                    continue
                kept = [w for w in si.on_wait
                        if not str(getattr(w, "ant_name", "")).startswith(pref)]
                if len(kept) != len(si.on_wait):
                    si.on_wait = kept


def _get_program():
    if "nc" not in _CACHE:
        _CACHE["nc"] = _build_program()
    return _CACHE["nc"]


def _make_in_maps(mk, mv, qk):
    f8 = ml_dtypes.float8_e4m3
    mkf = np.ascontiguousarray(mk.reshape(B, CK, N))
    mvf = np.ascontiguousarray(mv.reshape(B, CV, N))
    qkf = np.ascontiguousarray(qk.reshape(B, CK, P))
    in_maps = []
    for core in range(8):
        b, half = core // 2, core % 2
        n0, n1 = half * NHALF, (half + 1) * NHALF
        mk_c = mkf[b, :, n0:n1].astype(ml_dtypes.bfloat16)   # [64, 6480]
        # zero-pad contraction dim to 128 and n to NPAD
        mk_t = np.zeros((128, NT, 128), dtype=ml_dtypes.bfloat16)
        mk_t[:CK].reshape(CK, NPAD)[:, :NHALF] = mk_c
        qk_c = np.zeros((128, P), dtype=ml_dtypes.bfloat16)
        qk_c[:CK] = qkf[b].astype(ml_dtypes.bfloat16)
        mvt = np.zeros((NPAD, CV), dtype=f8)
        mvt[:NHALF] = mvf[b, :, n0:n1].T.astype(f8)
        # DoubleRow pair layout: [128, pair, plane, CV],
        # elem (k, t, j, v) = mvT[(2t+j)*128 + k, v]
        mv8 = np.ascontiguousarray(
            mvt.reshape(NT2, 2, 128, CV).transpose(2, 0, 1, 3)
        )
        in_maps.append({"mk": np.ascontiguousarray(mk_t),
                        "qk": np.ascontiguousarray(qk_c),
                        "mv8": mv8})
    return in_maps


def _run(mk, mv, qk, qv, trace=False, **spmd_kwargs):
    nc = _get_program()
    in_maps = _make_in_maps(mk, mv, qk)
    res = run_bass_kernel_spmd(nc, in_maps, list(range(8)), trace=trace, **spmd_kwargs)
    out = np.empty((B, 2 * CV, P), dtype=np.float32)
    for b in range(B):
        m0, l0 = res.results[2 * b]["memT"], res.results[2 * b]["lsum"]
        m1, l1 = res.results[2 * b + 1]["memT"], res.results[2 * b + 1]["lsum"]
        lv = (l0 + l1).sum(axis=(0, 1))          # [P]
        out[b, :CV] = ((m0 + m1) / lv[:, None]).T
        out[b, CV:] = qv[b].reshape(CV, P)
    return out.reshape(B, 2 * CV, H, W), res


def kernel(mk, mv, qk, qv):
    out, _ = _run(np.asarray(mk), np.asarray(mv), np.asarray(qk), np.asarray(qv))
    return out
